# revision 4
# baseline (speedup 1.0000x reference)
"""BinaryLinear kernel for Trainium2, data-parallel over 8 NeuronCores.

Computes y = x @ (sign(W) * scale).T + b where
  sign(w) = +1 if w >= 0 else -1
  scale_o = max(mean_i |W[o,i]|, 1e-6)           (per output row)

Strategy
--------
- Shard batch (32768) across 8 cores -> 4096 rows/core; replicate weights.
- sign(W) and scale are computed on the HOST (scale from full-precision W,
  so that error source is gone entirely); the device only does matmuls and
  a fused scale*psum+bias epilogue.
- Mixed precision split of the 1024-long contraction, chosen so the
  measured max-rel error (1.79e-2) stays under the 2e-2 gate:
    k in [0,512):   x in fp8 e4m3, sign in fp8, matmul in DoubleRow perf
                    mode (two fp8 weights per PE cell -> K=256 per MM,
                    measured ~2x bf16 column rate)
    k in [512,1024): x and sign in bf16 (exact +-1), regular matmuls
  Per (batch-block, out-block) PSUM group: 2 DoubleRow MMs + 4 bf16 MMs
  accumulate f32 into one PSUM bank (measured 1.31us/group vs 1.73 all-
  bf16; 6 x 512-column streams is the accuracy-constrained PE floor).
- Skewed wave schedule: MM(unit u, out-block c) issues at wave u+c, so
  the 8 PSUM banks finish staggered ~1 wave apart instead of all in the
  final sweep.  Epilogues then never queue up and bank recycling never
  stalls the PE.  Block 0 instead runs its two DoubleRow sweeps first:
  the fp8 inputs (1MB) land well before the bf16 ones, giving the PE a
  ~4us runway that covers the remaining head DMA time.
- Head DMAs fan out over four engine queues (sync/scalar/vector/gpsimd)
  ordered so first-needed tiles lead each queue; later-stage x loads are
  queued behind them (per-queue FIFO) so they can't steal bandwidth from
  the critical head.  PE warm-up matmuls with no input deps cover the
  HAM clock-gate window (~3.4us) during the head so the real stream runs
  at 2.4 GHz from the start.
- Epilogues alternate DVE / ACT per out-chunk, halving the per-engine
  epilogue chain (ACT is otherwise idle) and shortening the kernel tail.
- Outputs collect in [128, 1024] bf16 tiles (two batch blocks) so stores
  are full-rate 2KB-per-partition DMAs; the last block stores per-c
  halves immediately after each staggered epilogue.
"""

import os
import sys
import types

for _p in ("/opt/trn_rl_repo",):
    if _p not in sys.path and os.path.isdir(_p):
        sys.path.append(_p)

import numpy as np
import ml_dtypes

import concourse.bacc as bacc
import concourse.mybir as mybir
from concourse import tile
from concourse.bass_utils import run_bass_kernel_spmd

N_CORES = 8
BATCH = 32768
SHARD = BATCH // N_CORES          # 4096 rows per core
IN = 1024
OUT = 1024
EPS = 1e-6
P = 128                           # SBUF partitions
NB = 512                          # moving free-dim per matmul
NBC = SHARD // NB                 # 8 batch blocks per core
OC = OUT // P                     # 8 output-feature chunks
K8 = 512                          # contraction columns done in fp8
JP = K8 // (2 * P)                # 2 DoubleRow k-pair units (256 each)
KB = (IN - K8) // P               # 4 bf16 k-chunk units (128 each)
NU = JP + KB                      # 6 accumulation units per group

F32 = mybir.dt.float32
BF16 = mybir.dt.bfloat16
FP8 = mybir.dt.float8e4
Alu = mybir.AluOpType
Act = mybir.ActivationFunctionType
DRMODE = mybir.MatmulPerfMode.DoubleRow

WARM_SMALL = 40                   # N=64 dummy MMs (fast burn)
WARM_BIG = 2                      # N=512 dummy MMs (slow burn)

# batch-block DMA stages: x8 pairs with block {0,1} up front (cheap, fp8),
# xb brings only block 0 up front; the rest follows per-queue FIFO.
X8_STAGES = [(0, 2), (2, 5), (5, 8)]
XB_STAGES = [(0, 1), (1, 5), (5, 8)]


def _install_trace_shim():
    """antenv.axon_hooks is absent in this image; recreate it so
    run_bass_kernel_spmd(trace=True) can capture NTFF profiles."""
    try:
        import antenv.axon_hooks  # noqa: F401
        return
    except ImportError:
        pass
    try:
        import trn_agent_boot.trn_boot as tb
        hooks = types.ModuleType("antenv.axon_hooks")
        hooks._hook = tb._ntff_profile_via_ctypes("/opt/axon/libaxon_pjrt.so")
        hooks.get_axon_ntff_profile_hook = lambda: hooks._hook
        hooks.set_axon_ntff_profile_hook = lambda h: setattr(hooks, "_hook", h)
        sys.modules["antenv.axon_hooks"] = hooks
        import concourse.bass_utils as bass_utils
        bass_utils.upload_artifacts = lambda tmpdir: f"file://{tmpdir}"
    except Exception:
        pass


def build_program():
    nc = bacc.Bacc("TRN2", target_bir_lowering=False, debug=False,
                   num_devices=N_CORES)

    # x8: fp8 part of x^T, row j*128+p holds k=256j+128i+p, laid out per
    # batch block as [nb][i][nn] so DMA slabs are contiguous and matmul
    # rhs slices are 3D [128, 2, 512] DoubleRow APs.
    x8_d = nc.dram_tensor("x8", [JP * P, NBC * 2 * NB], FP8,
                          kind="ExternalInput")
    xb_d = nc.dram_tensor("xb", [KB * P, SHARD], BF16, kind="ExternalInput")
    # st: fp8 sign(W)^T for k<512, row j*128+p, cols [i][o]
    st_d = nc.dram_tensor("st", [JP * P, 2 * OUT], FP8, kind="ExternalInput")
    # wt: bf16 sign(W)^T for k>=512, row m*128+p = k-512
    wt_d = nc.dram_tensor("wt", [KB * P, OUT], BF16, kind="ExternalInput")
    sc_d = nc.dram_tensor("sc", [OUT], F32, kind="ExternalInput")
    b_d = nc.dram_tensor("b", [OUT], F32, kind="ExternalInput")
    yt_d = nc.dram_tensor("yt", [OUT, SHARD], BF16, kind="ExternalOutput")

    with tile.TileContext(nc) as tc:
        with (
            tc.tile_pool(name="w_pool", bufs=1) as w_pool,
            tc.tile_pool(name="x_pool", bufs=1) as x_pool,
            tc.tile_pool(name="misc", bufs=1) as misc,
            tc.tile_pool(name="ps", bufs=8, space="PSUM") as ps_pool,
            tc.tile_pool(name="yo_pool", bufs=8) as yo_pool,
        ):
            # ---- PE warm-up (no input deps; covers the HAM window while
            # the head DMAs land so the real stream starts at 2.4 GHz)
            warm = misc.tile([P, NB], BF16, tag="warm", name="warm")
            nc.vector.memset(warm[:], 0.0)
            wps = ps_pool.tile([P, NB], F32, tag="ps", name="wps")
            for _ in range(WARM_SMALL):
                nc.tensor.matmul(wps[:, 0:64], warm[:, 0:P], warm[:, 0:64],
                                 start=True, stop=True)
            for _ in range(WARM_BIG):
                nc.tensor.matmul(wps[:], warm[:, 0:P], warm[:],
                                 start=True, stop=True)

            # ---- tiles
            st8 = [w_pool.tile([P, 2, OUT], FP8, tag=f"st{j}", name=f"st{j}")
                   for j in range(JP)]
            wt = [w_pool.tile([P, OUT], BF16, tag=f"wt{m}", name=f"wt{m}")
                  for m in range(KB)]
            x8t = [[x_pool.tile([P, 2 * (b1 - b0), NB], FP8,
                                tag=f"x8_{j}_{si}", name=f"x8_{j}_{si}")
                    for si, (b0, b1) in enumerate(X8_STAGES)]
                   for j in range(JP)]
            xbt = [[x_pool.tile([P, (b1 - b0) * NB], BF16,
                                tag=f"xb{m}_{si}", name=f"xb{m}_{si}")
                    for si, (b0, b1) in enumerate(XB_STAGES)]
                   for m in range(KB)]
            scol = misc.tile([P, OC], F32, tag="scol", name="scol")
            bcol = misc.tile([P, OC], F32, tag="bcol", name="bcol")

            def load_x8(j, si, eng):
                b0, b1 = X8_STAGES[si]
                eng.dma_start(x8t[j][si][:],
                              x8_d.ap()[j * P:(j + 1) * P,
                                        b0 * 2 * NB:b1 * 2 * NB])

            def load_xb(m, si, eng):
                b0, b1 = XB_STAGES[si]
                eng.dma_start(xbt[m][si][:],
                              xb_d.ap()[m * P:(m + 1) * P, b0 * NB:b1 * NB])

            # ---- head DMAs across the three DMA-capable queues (sync /
            # gpsimd / scalar); first-needed tiles lead each queue
            # (per-queue FIFO guarantees they transfer first).
            nc.sync.dma_start(st8[0][:], st_d.ap()[0:P, :])
            load_x8(0, 0, nc.gpsimd)
            load_x8(1, 0, nc.scalar)
            nc.gpsimd.dma_start(st8[1][:], st_d.ap()[P:2 * P, :])
            nc.gpsimd.dma_start(scol[:], sc_d.ap().rearrange("(c p) -> p c", p=P))
            nc.gpsimd.dma_start(bcol[:], b_d.ap().rearrange("(c p) -> p c", p=P))
            for m, eng in zip(range(KB), (nc.sync, nc.gpsimd, nc.sync, nc.scalar)):
                eng.dma_start(wt[m][:], wt_d.ap()[m * P:(m + 1) * P, :])
            for m, eng in zip(range(KB), (nc.sync, nc.gpsimd, nc.sync, nc.scalar)):
                load_xb(m, 0, eng)

            # later stages: queued behind the head (per-queue FIFO keeps
            # them from stealing bandwidth); scalar's are early so its
            # queue is clear for ACT epilogues + store dispatches.
            load_x8(0, 1, nc.scalar)
            load_x8(1, 1, nc.scalar)
            for m in range(KB):
                load_xb(m, 1, (nc.sync, nc.gpsimd)[m % 2])
            load_x8(0, 2, nc.sync)
            load_x8(1, 2, nc.gpsimd)
            for m in range(KB):
                load_xb(m, 2, (nc.sync, nc.gpsimd)[m % 2])

            def stage_of(stages, n):
                for si, (b0, b1) in enumerate(stages):
                    if b0 <= n < b1:
                        return si, n - b0
                raise AssertionError(n)

            def rhs_for(u, n):
                if u < JP:
                    si, ln = stage_of(X8_STAGES, n)
                    return x8t[u][si][:, 2 * ln:2 * ln + 2, :]
                si, ln = stage_of(XB_STAGES, n)
                return xbt[u - JP][si][:, ln * NB:(ln + 1) * NB]

            yo_cur = [None] * OC

            def epilogue(n, c, ps):
                half = n % 2
                if half == 0:
                    yo_cur[c] = yo_pool.tile([P, 2 * NB], BF16, tag="yo",
                                             name=f"yo{n}_{c}")
                yo = yo_cur[c]
                dst = yo[:, half * NB:(half + 1) * NB]
                if c % 2 == 0:
                    nc.vector.tensor_scalar(dst, ps[:], scol[:, c:c + 1],
                                            bcol[:, c:c + 1], Alu.mult, Alu.add)
                else:
                    nc.scalar.activation(dst, ps[:], Act.Identity,
                                         bias=bcol[:, c:c + 1],
                                         scale=scol[:, c:c + 1])
                if n == NBC - 2:
                    # penultimate block: store its half immediately so it
                    # overlaps the last block's compute
                    nc.scalar.dma_start(
                        yt_d.ap()[c * P:(c + 1) * P, n * NB:(n + 1) * NB],
                        yo[:, 0:NB])
                elif n == NBC - 1:
                    # last block: per-c half stores fire as each staggered
                    # epilogue completes -> short kernel tail
                    eng = nc.sync if c % 2 == 1 else nc.scalar
                    eng.dma_start(
                        yt_d.ap()[c * P:(c + 1) * P, n * NB:(n + 1) * NB],
                        yo[:, NB:2 * NB])
                elif half == 1:
                    eng = nc.scalar if c % 2 == 1 else nc.sync
                    eng.dma_start(
                        yt_d.ap()[c * P:(c + 1) * P,
                                  (n - 1) * NB:(n + 1) * NB],
                        yo[:])

            def mm(u, c, n, ps):
                if u < JP:
                    nc.tensor.matmul(ps[:], st8[u][:, :, c * P:(c + 1) * P],
                                     rhs_for(u, n), start=(u == 0), stop=False,
                                     perf_mode=DRMODE)
                else:
                    nc.tensor.matmul(ps[:], wt[u - JP][:, c * P:(c + 1) * P],
                                     rhs_for(u, n), start=False,
                                     stop=(u == NU - 1))

            # ---- main loop: skewed waves.  MM(unit u, out-chunk c) goes
            # at wave u+c; each bank's 6-MM accumulation finishes one wave
            # after the previous bank's, so epilogues stagger and PSUM
            # banks are long free before block n+1 reuses them.  Block 0
            # runs its DoubleRow sweeps first (fp8 data lands first) with
            # the skew applied to the bf16 sweeps only.
            for n in range(NBC):
                yps = [ps_pool.tile([P, NB], F32, tag="ps", name=f"yp{n}_{c}")
                       for c in range(OC)]
                if n == 0:
                    for u in range(JP):
                        for c in range(OC):
                            mm(u, c, n, yps[c])
                    for wv in range(KB + OC - 1):
                        for c in range(OC):
                            u = JP + wv - c
                            if JP <= u < NU:
                                mm(u, c, n, yps[c])
                                if u == NU - 1:
                                    epilogue(n, c, yps[c])
                else:
                    for wv in range(NU + OC - 1):
                        for c in range(OC):
                            u = wv - c
                            if 0 <= u < NU:
                                mm(u, c, n, yps[c])
                                if u == NU - 1:
                                    epilogue(n, c, yps[c])

    nc.compile()
    return nc


_NC = None


def _get_program():
    global _NC
    if _NC is None:
        _NC = build_program()
    return _NC


def kernel(x: np.ndarray, W: np.ndarray, b: np.ndarray) -> np.ndarray:
    assert x.shape == (BATCH, IN) and W.shape == (OUT, IN) and b.shape == (OUT,)
    nc = _get_program()

    Wf = np.asarray(W, dtype=np.float32)
    sgnT = np.where(Wf >= 0, np.float32(1.0), np.float32(-1.0)).T  # [in, out]
    st_pack = np.ascontiguousarray(
        sgnT[:K8].reshape(JP, 2, P, OUT).transpose(0, 2, 1, 3)
        .reshape(JP * P, 2 * OUT)).astype(ml_dtypes.float8_e4m3)
    wt_pack = np.ascontiguousarray(sgnT[K8:]).astype(ml_dtypes.bfloat16)
    sc = np.maximum(np.abs(Wf).mean(axis=1), EPS).astype(np.float32)
    b32 = np.ascontiguousarray(np.asarray(b, dtype=np.float32))

    in_maps = []
    for c in range(N_CORES):
        xt = x[c * SHARD:(c + 1) * SHARD].T      # [in, n] view
        x8 = xt[:K8].astype(ml_dtypes.float8_e4m3)
        x8 = np.ascontiguousarray(
            x8.reshape(JP, 2, P, NBC, NB).transpose(0, 2, 3, 1, 4)
            .reshape(JP * P, NBC * 2 * NB))
        xb = np.ascontiguousarray(xt[K8:]).astype(ml_dtypes.bfloat16)
        in_maps.append({"x8": x8, "xb": xb, "st": st_pack, "wt": wt_pack,
                        "sc": sc, "b": b32})

    trace = bool(int(os.environ.get("BINLIN_TRACE", "0")))
    if trace:
        _install_trace_shim()
    res = run_bass_kernel_spmd(nc, in_maps, core_ids=list(range(N_CORES)),
                               trace=trace)
    if trace and res.exec_time_ns is not None:
        print(f"HW exec time: {res.exec_time_ns} ns", flush=True)

    y = np.empty((BATCH, OUT), dtype=np.float32)
    for c in range(N_CORES):
        y[c * SHARD:(c + 1) * SHARD] = res.results[c]["yt"].T.astype(np.float32)
    return y
